# revision 12
# baseline (speedup 1.0000x reference)
"""Causal self-attention (B=2, T=2048, C=1024, H=16) on 8 TRN2 NeuronCores.

Sharding: core c -> batch b = c//4, head group hg = c%4 (4 heads/core).
Each core computes QKV for its 4 heads (column-parallel), causal attention,
and a row-parallel partial output projection [T, C] in bf16. The host sums
the 4 partials per batch and adds the analytically-folded biases.

Schedule: a software-pipelined attention j-loop (S matmul + exp run 2 steps
ahead of PV), with QKV/V/proj matmuls deficit-paced as fillers between
attention steps so the PE stays fed while the Scalar engine (exp) streams.
Causal mask is a post-exp multiply by a 0/1 triangle on GpSimd.
"""

import sys

if "/opt/trn_rl_repo" not in sys.path:
    sys.path.insert(0, "/opt/trn_rl_repo")

import numpy as np
import ml_dtypes
from contextlib import ExitStack

import concourse.bass as bass
import concourse.mybir as mybir
import concourse.tile as tile
from concourse import bacc, bass_utils
from concourse.bass import ds, ts


BF = mybir.dt.bfloat16
F32 = mybir.dt.float32

B, T, C = 2, 2048, 1024
H, DK = 16, 64
P = 128
KC = C // P          # 8 contraction chunks over C
NTG = T // 512       # 4 t-groups of 512
NTJ = T // 128       # 16 t-chunks of 128
HPC = 4              # heads per core
VS = 66              # vaug per-head stride (cols 0-63 V, 64 ones, 65 pad)

# interleaved (m, g) order: m0 runs ahead so m1's QK weights can stream in;
# the tiny (1, 0) group last keeps the kernel tail short
ORDER = [(0, 0), (0, 1), (1, 1), (0, 2), (1, 2), (0, 3), (1, 3), (1, 0)]

# module-level knobs for test harness
TRACE = False
TRACE_KWARGS = {}
LAST_RESULTS = None


def _emit(ctx, tc, aps):
    nc = tc.nc
    xt, wq, wk, wv, bq, bk, wp, tri, out = (
        aps["xt"], aps["wq"], aps["wk"], aps["wv"], aps["bq"], aps["bk"],
        aps["wp"], aps["tri"], aps["out"],
    )

    consts = ctx.enter_context(tc.tile_pool(name="consts", bufs=1))
    bigs = ctx.enter_context(tc.tile_pool(name="bigs", bufs=1))
    temps = ctx.enter_context(tc.tile_pool(name="temps", bufs=2))
    ppool = ctx.enter_context(tc.tile_pool(name="ppool", bufs=4))
    psum = ctx.enter_context(tc.tile_pool(name="psum", bufs=1, space="PSUM"))
    dpool = ctx.enter_context(tc.tile_pool(name="dpool", bufs=4, space="DRAM"))

    # ---- SBUF residents ----
    wqs = consts.tile([P, 2, KC, P], BF)
    wks = consts.tile([P, 2, KC, P], BF)
    wvs = consts.tile([P, KC, 2 * P], BF)
    wps = consts.tile([P, 2, C], BF)
    bqs = consts.tile([P, 2], F32)
    bks = consts.tile([P, 2], F32)
    tris = consts.tile([P, 2, P], BF)
    xts = bigs.tile([P, KC, T], BF)
    qt = bigs.tile([P, 2, T], BF)
    kt = bigs.tile([P, 2, T], BF)
    vaug = bigs.tile([P, NTJ, HPC * VS], BF)
    vaug4 = vaug.rearrange("p t (h c) -> p t h c", c=VS)
    yts = [bigs.tile([P, T], BF, name=f"yt{m}") for m in range(2)]

    # ---- input DMAs, ordered so the first QKV matmuls unblock earliest.
    # Two queues (sync / gpsimd); each dma_start stripes over the HW engines.
    nc.sync.dma_start(out=bqs, in_=bq.rearrange("(m p) -> p m", p=P))
    nc.gpsimd.dma_start(out=bks, in_=bk.rearrange("(m p) -> p m", p=P))
    nc.gpsimd.dma_start(out=tris, in_=tri)
    nc.sync.dma_start(out=wqs[:, 0, 0:4], in_=wq[0, :, 0:4])
    nc.gpsimd.dma_start(out=wks[:, 0, 0:4], in_=wk[0, :, 0:4])
    nc.sync.dma_start(out=xts[:, 0:4, 0:512], in_=xt[0, :, 0:4, :])
    nc.gpsimd.dma_start(out=wks[:, 0, 4:8], in_=wk[0, :, 4:8])
    nc.sync.dma_start(out=wqs[:, 0, 4:8], in_=wq[0, :, 4:8])
    nc.gpsimd.dma_start(out=wvs, in_=wv)
    nc.sync.dma_start(out=xts[:, 4:8, 0:512], in_=xt[0, :, 4:8, :])
    nc.sync.dma_start(out=xts[:, 0:4, 512:1024], in_=xt[1, :, 0:4, :])
    nc.gpsimd.dma_start(out=xts[:, 4:8, 512:1024], in_=xt[1, :, 4:8, :])
    nc.gpsimd.dma_start(out=wks[:, 1], in_=wk[1])
    nc.sync.dma_start(out=wqs[:, 1], in_=wq[1])
    nc.sync.dma_start(out=xts[:, :, ts(2, 512)], in_=xt[2])
    nc.gpsimd.dma_start(out=xts[:, :, ts(3, 512)], in_=xt[3])
    nc.sync.dma_start(out=wps, in_=wp)

    nc.vector.memset(vaug4[:, :, :, DK : DK + 1], 1.0)

    # ---- filler units (QKV / V / proj work woven between attention steps) ----
    def qk_unit(m, dst, tg):
        wsrc, bsrc, dstt = (
            (wqs, bqs, qt) if dst == "q" else (wks, bks, kt)
        )
        pq = psum.tile([P, 512], F32, tag="mm", bufs=2, name="pq")
        for k in range(KC):
            nc.tensor.matmul(
                pq,
                lhsT=wsrc[:, m, k, :],
                rhs=xts[:, k, ts(tg, 512)],
                start=(k == 0),
                stop=(k == KC - 1),
            )
        nc.vector.tensor_add(
            out=dstt[:, m, ts(tg, 512)],
            in0=pq,
            in1=bsrc[:, m : m + 1].to_broadcast([P, 512]),
        )

    def v_unit(tj):
        pv = psum.tile([P, 512], F32, tag="mm", bufs=2, name="pv")
        for k in range(KC):
            nc.tensor.matmul(
                pv[:, : 2 * P],
                lhsT=xts[:, k, ts(tj, P)],
                rhs=wvs[:, k, :],
                start=(k == 0),
                stop=(k == KC - 1),
            )
        nc.vector.tensor_copy(
            out=vaug4[:, tj, :, 0:DK],
            in_=pv[:, : 2 * P].rearrange("p (h d) -> p h d", d=DK),
        )

    def proj_unit(tj, msel=None):
        pps = [
            psum.tile([P, 512], F32, tag="mm", bufs=2, name=f"pp{n}")
            for n in range(2)
        ]
        kcs = (0, 1) if msel is None else (msel,)
        for kc in kcs:
            for n in range(2):
                nc.tensor.matmul(
                    pps[n],
                    lhsT=yts[kc][:, ts(tj, P)],
                    rhs=wps[:, kc, ts(n, 512)],
                    start=(kc == kcs[0]),
                    stop=(kc == kcs[-1]),
                )
        for n in range(2):
            ostg = temps.tile([P, 512], BF, tag="ostg", bufs=4, name="ostg")
            nc.vector.tensor_copy(out=ostg, in_=pps[n])
            eng = nc.sync if n == 0 else nc.gpsimd
            if msel == 1:
                # second head-pair accumulates onto the first's partial
                # (accum is software-DGE only -> gpsimd queue)
                eng = nc.gpsimd
                eng.dma_start(
                    out=out[ts(tj, P), ts(n, 512)],
                    in_=ostg,
                    accum_op=mybir.AluOpType.add,
                )
            else:
                eng.dma_start(out=out[ts(tj, P), ts(n, 512)], in_=ostg)

    # FIFO of (name, fn, pe_ns); forced-emitted for deps, else deficit-paced
    fillers = (
        [(("v", tj), (lambda t: lambda: v_unit(t))(tj), 853) for tj in range(4)]
        + [(("qk", 0, d, 1), (lambda d_: lambda: qk_unit(0, d_, 1))(d), 1707)
           for d in ("q", "k")]
        + [(("v", tj), (lambda t: lambda: v_unit(t))(tj), 853)
           for tj in range(4, 8)]
        + [(("qk", 1, "k", 0), lambda: qk_unit(1, "k", 0), 1707)]
        + [(("qk", 1, d, 1), (lambda d_: lambda: qk_unit(1, d_, 1))(d), 1707)
           for d in ("q", "k")]
        + [(("qk", 0, d, 2), (lambda d_: lambda: qk_unit(0, d_, 2))(d), 1707)
           for d in ("q", "k")]
        + [(("v", tj), (lambda t: lambda: v_unit(t))(tj), 853)
           for tj in range(8, 12)]
        + [(("qk", 1, d, 2), (lambda d_: lambda: qk_unit(1, d_, 2))(d), 1707)
           for d in ("q", "k")]
        + [(("qk", 0, d, 3), (lambda d_: lambda: qk_unit(0, d_, 3))(d), 1707)
           for d in ("q", "k")]
        + [(("v", tj), (lambda t: lambda: v_unit(t))(tj), 853)
           for tj in range(12, 16)]
        + [(("qk", 1, d, 3), (lambda d_: lambda: qk_unit(1, d_, 3))(d), 1707)
           for d in ("q", "k")]
        + [(("qk", 1, "q", 0), lambda: qk_unit(1, "q", 0), 1707)]
    )
    emitted = set()
    state = {"deficit": 0.0}

    def pop_filler():
        name, fn, cost = fillers.pop(0)
        fn()
        emitted.add(name)
        state["deficit"] -= cost

    def force(names):
        while any(n not in emitted for n in names):
            pop_filler()

    # bootstrap: first attention group's QK
    qk_unit(0, "q", 0)
    emitted.add(("qk", 0, "q", 0))
    qk_unit(0, "k", 0)
    emitted.add(("qk", 0, "k", 0))

    # ---- attention steps ----
    steps = []
    for m, g in ORDER:
        njc = 4 * g + 4
        for j in range(njc):
            steps.append((m, g, j, njc))

    po_tiles = {}       # (m, g) -> [po_h0, po_h1]
    pt_tiles = {}       # step idx -> pt2 tile
    finalized = set()   # g values with one m done

    def emit_S(i):
        m, g, j, njc = steps[i]
        jrel = j - 4 * g
        band = jrel >= 0
        ncols = 512 - 128 * jrel if band else 512
        qoff = g * 512 + (128 * jrel if band else 0)
        ps2 = psum.tile([P, 1024], F32, tag="s", bufs=2, name="ps2")
        for h in range(2):
            nc.tensor.matmul(
                ps2[:, h * 512 : h * 512 + ncols],
                lhsT=kt[h * DK : (h + 1) * DK, m, ts(j, P)],
                rhs=qt[h * DK : (h + 1) * DK, m, ds(qoff, ncols)],
                start=True,
                stop=True,
                tile_position=(h * DK, 0),
            )
        pt2 = ppool.tile([P, 1024], BF, tag="pt", bufs=4, name="pt2")
        psv = ps2.rearrange("p (h c) -> p h c", h=2)
        ptv = pt2.rearrange("p (h c) -> p h c", h=2)
        nc.scalar.activation(
            ptv[:, :, 0:ncols],
            psv[:, :, 0:ncols],
            mybir.ActivationFunctionType.Exp,
        )
        if band:
            # zero the masked (upper-tri) part of the diagonal 128-block
            nc.vector.tensor_mul(
                out=ptv[:, :, 0:P], in0=ptv[:, :, 0:P], in1=tris
            )
        pt_tiles[i] = pt2

    def emit_PV(i):
        m, g, j, njc = steps[i]
        jrel = j - 4 * g
        band = jrel >= 0
        ncols = 512 - 128 * jrel if band else 512
        if j == 0:
            po_tiles[(m, g)] = [
                psum.tile([DK + 1, 512], F32, tag=f"o{h}", bufs=1, name=f"po{h}")
                for h in range(2)
            ]
        po = po_tiles[(m, g)]
        pt2 = pt_tiles.pop(i)
        co = 128 * jrel if band else 0
        for h in range(2):
            nc.tensor.matmul(
                po[h][:, co : co + ncols],
                lhsT=vaug4[:, j, 2 * m + h, : DK + 1],
                rhs=pt2[:, h * 512 : h * 512 + ncols],
                start=(j == 0),
                stop=(j == njc - 1),
                skip_group_check=True,
            )
        if j == njc - 1:
            finalize(m, g)

    def finalize(m, g):
        po = po_tiles.pop((m, g))
        rbl = temps.tile([DK, 1024], F32, tag="rbl", bufs=2, name="rbl")
        ocs = []
        for h in range(2):
            oc = temps.tile([DK + 1, 512], F32, tag=f"oc{h}", bufs=2, name="oc")
            nc.vector.tensor_copy(out=oc, in_=po[h])
            ocs.append(oc)
            dscr = dpool.tile([512], F32, tag=f"dscr{h}", bufs=4, name="dscr")
            nc.sync.dma_start(out=dscr, in_=oc[DK : DK + 1, :])
            nc.gpsimd.dma_start(
                out=rbl[:, ts(h, 512)],
                in_=bass.AP(
                    tensor=dscr.tensor,
                    offset=dscr.offset,
                    ap=[[0, DK]] + list(dscr.ap),
                ),
            )
        rb = temps.tile([DK, 1024], F32, tag="rb", bufs=2, name="rb")
        nc.vector.reciprocal_approx_fast(out=rb, in_=rbl)
        for h in range(2):
            stg = temps.tile([DK, 512], BF, tag=f"stg{h}", bufs=2, name="stg")
            nc.gpsimd.tensor_mul(
                out=stg, in0=ocs[h][0:DK, :], in1=rb[:, ts(h, 512)]
            )
            nc.gpsimd.dma_start(
                out=yts[m][h * DK : (h + 1) * DK, ts(g, 512)], in_=stg
            )
        # g0/g3 bracket the schedule: emit per-m proj halves immediately so
        # they fill the pipeline instead of piling up at the kernel tail.
        if g in (0, 3):
            for tj in range(4 * g, 4 * g + 4):
                fillers.append(
                    (("projm", m, tj),
                     (lambda m_, t: lambda: proj_unit(t, m_))(m, tj), 427)
                )
        elif g in finalized:
            for tj in range(4 * g, 4 * g + 4):
                fillers.append(
                    (("proj", tj), (lambda t: lambda: proj_unit(t))(tj), 853)
                )
        else:
            finalized.add(g)
        if g not in finalized:
            finalized.add(g)

    def s_deps(i):
        m, g, j, njc = steps[i]
        return [("qk", m, "q", g)] + [("qk", m, "k", tg) for tg in range(g + 1)]

    def pv_deps(i):
        m, g, j, njc = steps[i]
        return [("v", j)]

    LOOKAHEAD = 2
    for i in range(len(steps)):
        m, g, j, njc = steps[i]
        force(s_deps(i))
        emit_S(i)
        jrel = j - 4 * g
        ncols = 512 - 128 * jrel if jrel >= 0 else 512
        state["deficit"] += (2 * ncols * 1.23 + 90) - (4 * ncols * 0.4167)
        state["deficit"] = max(-4000.0, min(9000.0, state["deficit"]))
        while state["deficit"] > 900 and fillers:
            pop_filler()
        if i >= LOOKAHEAD:
            force(pv_deps(i - LOOKAHEAD))
            emit_PV(i - LOOKAHEAD)
    for i in range(len(steps) - LOOKAHEAD, len(steps)):
        force(pv_deps(i))
        emit_PV(i)
    while fillers:
        pop_filler()


def _dedupe_ldweights(nc):
    """Drop an InstLdweights when the immediately-preceding PE weight load in
    the scheduled stream is byte-identical (only matmuls in between — they
    don't disturb the stationary operand)."""
    removed = 0
    for f in nc.m.functions:
        for bb in f.blocks:
            insts = list(bb.instructions)
            last_sig = None
            to_remove = []
            for inst in insts:
                tn = type(inst).__name__
                if tn == "InstLdweights":
                    si = inst.sync_info
                    has_sync = si is not None and (
                        list(si.on_wait) or list(si.on_update)
                    )
                    sig = (
                        str(inst.ins[0]),
                        str(inst.tile_position),
                        str(inst.tile_size),
                        str(inst.perf_mode),
                        str(inst.is_transpose),
                    )
                    if sig == last_sig and not has_sync:
                        to_remove.append(inst)
                        continue
                    last_sig = sig
                elif tn == "InstMatmult":
                    continue
                elif getattr(inst, "engine", None) == mybir.EngineType.PE:
                    last_sig = None
            for inst in to_remove:
                bb.instructions.remove(inst)
                removed += 1
    return removed


_NC_CACHE = None


def build():
    global _NC_CACHE
    if _NC_CACHE is not None:
        return _NC_CACHE
    nc = bacc.Bacc("TRN2", target_bir_lowering=False, debug=False, num_devices=8)
    aps = {
        "xt": nc.dram_tensor("xt", [NTG, P, KC, 512], BF, kind="ExternalInput").ap(),
        "wq": nc.dram_tensor("wq", [2, P, KC, P], BF, kind="ExternalInput").ap(),
        "wk": nc.dram_tensor("wk", [2, P, KC, P], BF, kind="ExternalInput").ap(),
        "wv": nc.dram_tensor("wv", [P, KC, 2 * P], BF, kind="ExternalInput").ap(),
        "bq": nc.dram_tensor("bq", [2 * P], F32, kind="ExternalInput").ap(),
        "bk": nc.dram_tensor("bk", [2 * P], F32, kind="ExternalInput").ap(),
        "wp": nc.dram_tensor("wp", [P, 2, C], BF, kind="ExternalInput").ap(),
        "tri": nc.dram_tensor("tri", [P, 2, P], BF, kind="ExternalInput").ap(),
        "out": nc.dram_tensor("out", [T, C], BF, kind="ExternalOutput").ap(),
    }
    with tile.TileContext(nc) as tc:
        with ExitStack() as ctx:
            _emit(ctx, tc, aps)
    _dedupe_ldweights(nc)
    nc.compile()
    _NC_CACHE = nc
    return nc


def make_in_maps(x, Wqkv, bqkv, Wproj):
    """Host-side sharding/layout prep. Returns per-core input dicts."""
    bf = ml_dtypes.bfloat16
    scale = np.float32(1.0 / np.sqrt(DK))
    # post-exp multiplicative causal mask for the diagonal 128-block:
    # tri[k, h, q] = 1 if q >= k else 0 (same for both heads)
    tri1 = (np.arange(P)[None, :] >= np.arange(P)[:, None]).astype(np.float32)
    triv = np.ascontiguousarray(
        np.broadcast_to(tri1[:, None, :], (P, 2, P))
    ).astype(bf)

    def lay_w(w):  # [C, 256] -> [m, p, k, 128] linear
        return np.ascontiguousarray(
            w.reshape(KC, P, 2, P).transpose(2, 1, 0, 3)
        ).astype(bf)

    def lay_x(xb):  # [T, C] -> [tg, p, k, 512] linear
        xt = xb.T  # [C, T]
        return np.ascontiguousarray(
            xt.reshape(KC, P, NTG, 512).transpose(2, 1, 0, 3)
        ).astype(bf)

    xts = [lay_x(x[b]) for b in range(B)]
    in_maps = []
    for c in range(8):
        b, hg = divmod(c, 4)
        lo = hg * HPC * DK
        sl = slice(lo, lo + HPC * DK)
        in_maps.append(
            {
                "xt": xts[b],
                "wq": lay_w(Wqkv[:, 0 * C :][:, sl] * scale),
                "wk": lay_w(Wqkv[:, 1 * C :][:, sl]),
                "wv": np.ascontiguousarray(
                    Wqkv[:, 2 * C :][:, sl].reshape(KC, P, 2 * P).transpose(1, 0, 2)
                ).astype(bf),
                "bq": np.ascontiguousarray(bqkv[0 * C :][sl] * scale).astype(np.float32),
                "bk": np.ascontiguousarray(bqkv[1 * C :][sl]).astype(np.float32),
                "wp": np.ascontiguousarray(
                    Wproj[sl, :].reshape(2, P, C).transpose(1, 0, 2)
                ).astype(bf),
                "tri": triv,
            }
        )
    return in_maps


def gather(outs, bqkv, Wproj, bproj):
    """Sum per-core bf16 partials per batch; fold V-bias + proj-bias."""
    bv = bqkv[2 * C :].astype(np.float32)
    bp_eff = (bproj.astype(np.float32) + bv @ Wproj.astype(np.float32)).astype(
        np.float32
    )
    y = np.empty((B, T, C), np.float32)
    for b in range(B):
        acc = outs[b * 4 + 0].astype(np.float32)
        for hg in range(1, 4):
            acc = acc + outs[b * 4 + hg].astype(np.float32)
        y[b] = acc + bp_eff[None, :]
    return y


def kernel(x, Wqkv, bqkv, Wproj, bproj):
    global LAST_RESULTS
    x = np.asarray(x, dtype=np.float32)
    Wqkv = np.asarray(Wqkv, dtype=np.float32)
    bqkv = np.asarray(bqkv, dtype=np.float32)
    Wproj = np.asarray(Wproj, dtype=np.float32)
    bproj = np.asarray(bproj, dtype=np.float32)

    nc = build()
    in_maps = make_in_maps(x, Wqkv, bqkv, Wproj)
    try:
        res = bass_utils.run_bass_kernel_spmd(
            nc,
            in_maps,
            core_ids=list(range(8)),
            trace=TRACE,
            **TRACE_KWARGS,
        )
    except Exception:
        if not TRACE:
            raise
        import traceback

        traceback.print_exc()
        print("traced run failed; retrying without trace", file=sys.stderr)
        res = bass_utils.run_bass_kernel_spmd(nc, in_maps, core_ids=list(range(8)))
    LAST_RESULTS = res
    outs = [res.results[c]["out"] for c in range(8)]
    return gather(outs, bqkv, Wproj, bproj)


# revision 20
# speedup vs baseline: 1.1031x; 1.1031x over previous
"""Causal self-attention (B=2, T=2048, C=1024, H=16) on 8 TRN2 NeuronCores.

Sharding: core c -> batch b = c//4, head group hg = c%4 (4 heads/core).
Each core computes QKV for its 4 heads (column-parallel), causal attention,
and a row-parallel partial output projection [T, C] in bf16. The host sums
the 4 partials per batch and adds the analytically-folded biases.

Schedule: a software-pipelined attention j-loop (S matmul + exp run 2 steps
ahead of PV), with QKV/V/proj matmuls deficit-paced as fillers between
attention steps so the PE stays fed while the Scalar engine (exp) streams.
Causal mask is a post-exp multiply by a 0/1 triangle on GpSimd.
"""

import sys

if "/opt/trn_rl_repo" not in sys.path:
    sys.path.insert(0, "/opt/trn_rl_repo")

import numpy as np
import ml_dtypes
from contextlib import ExitStack

import concourse.bass as bass
import concourse.mybir as mybir
import concourse.tile as tile
from concourse import bacc, bass_utils
from concourse.bass import ds, ts


BF = mybir.dt.bfloat16
F32 = mybir.dt.float32

B, T, C = 2, 2048, 1024
H, DK = 16, 64
P = 128
KC = C // P          # 8 contraction chunks over C
NTG = T // 512       # 4 t-groups of 512
NTJ = T // 128       # 16 t-chunks of 128
HPC = 4              # heads per core
VS = 66              # vaug per-head stride (cols 0-63 V, 64 ones, 65 pad)

# interleaved (m, g) order: m0 runs ahead so m1's QK weights can stream in
ORDER = [(0, 0), (0, 1), (1, 0), (1, 1), (0, 2), (1, 2), (0, 3), (1, 3)]

# module-level knobs for test harness
TRACE = False
TRACE_KWARGS = {}
LAST_RESULTS = None


def _emit(ctx, tc, aps):
    nc = tc.nc
    xt, wq, wk, wv, bq, bk, wp, tri, out, outx = (
        aps["xt"], aps["wq"], aps["wk"], aps["wv"], aps["bq"], aps["bk"],
        aps["wp"], aps["tri"], aps["out"], aps["outx"],
    )

    consts = ctx.enter_context(tc.tile_pool(name="consts", bufs=1))
    bigs = ctx.enter_context(tc.tile_pool(name="bigs", bufs=1))
    temps = ctx.enter_context(tc.tile_pool(name="temps", bufs=2))
    ppool = ctx.enter_context(tc.tile_pool(name="ppool", bufs=4))
    psum = ctx.enter_context(tc.tile_pool(name="psum", bufs=1, space="PSUM"))
    dpool = ctx.enter_context(tc.tile_pool(name="dpool", bufs=4, space="DRAM"))

    # ---- SBUF residents ----
    wqs = consts.tile([P, 2, KC, P], BF)
    wks = consts.tile([P, 2, KC, P], BF)
    wvs = consts.tile([P, KC, 2 * P], BF)
    wps = consts.tile([P, 2, C], BF)
    bqs = consts.tile([P, 2], F32)
    bks = consts.tile([P, 2], F32)
    tris = consts.tile([P, 2, P], BF)
    xts = bigs.tile([P, KC, T], BF)
    qt = bigs.tile([P, 2, T], BF)
    kt = bigs.tile([P, 2, T], BF)
    vaug = bigs.tile([P, NTJ, HPC * VS], BF)
    vaug4 = vaug.rearrange("p t (h c) -> p t h c", c=VS)
    yts = [bigs.tile([P, T], BF, name=f"yt{m}") for m in range(2)]

    # ---- input DMAs, ordered so the first QKV matmuls unblock earliest.
    # Two queues (sync / gpsimd); each dma_start stripes over the HW engines.
    nc.sync.dma_start(out=bqs, in_=bq.rearrange("(m p) -> p m", p=P))
    nc.gpsimd.dma_start(out=bks, in_=bk.rearrange("(m p) -> p m", p=P))
    nc.gpsimd.dma_start(out=tris, in_=tri)
    nc.sync.dma_start(out=wqs[:, 0, 0:4], in_=wq[0, :, 0:4])
    nc.gpsimd.dma_start(out=wks[:, 0, 0:4], in_=wk[0, :, 0:4])
    nc.sync.dma_start(out=xts[:, 0:4, 0:512], in_=xt[0, :, 0:4, :])
    nc.gpsimd.dma_start(out=wks[:, 0, 4:8], in_=wk[0, :, 4:8])
    nc.sync.dma_start(out=wqs[:, 0, 4:8], in_=wq[0, :, 4:8])
    nc.gpsimd.dma_start(out=wvs, in_=wv)
    nc.sync.dma_start(out=xts[:, 4:8, 0:512], in_=xt[0, :, 4:8, :])
    nc.sync.dma_start(out=xts[:, 0:4, 512:1024], in_=xt[1, :, 0:4, :])
    nc.gpsimd.dma_start(out=xts[:, 4:8, 512:1024], in_=xt[1, :, 4:8, :])
    nc.gpsimd.dma_start(out=wks[:, 1], in_=wk[1])
    nc.sync.dma_start(out=wqs[:, 1], in_=wq[1])
    nc.sync.dma_start(out=xts[:, :, ts(2, 512)], in_=xt[2])
    nc.gpsimd.dma_start(out=xts[:, :, ts(3, 512)], in_=xt[3])
    nc.sync.dma_start(out=wps, in_=wp)

    nc.vector.memset(vaug4[:, :, :, DK : DK + 1], 1.0)

    # ---- filler units (QKV / V / proj work woven between attention steps) ----
    def qk_unit(m, dst, tg):
        wsrc, bsrc, dstt = (
            (wqs, bqs, qt) if dst == "q" else (wks, bks, kt)
        )
        pq = psum.tile([P, 512], F32, tag="mm", bufs=2, name="pq")
        for k in range(KC):
            nc.tensor.matmul(
                pq,
                lhsT=wsrc[:, m, k, :],
                rhs=xts[:, k, ts(tg, 512)],
                start=(k == 0),
                stop=(k == KC - 1),
            )
        nc.vector.tensor_add(
            out=dstt[:, m, ts(tg, 512)],
            in0=pq,
            in1=bsrc[:, m : m + 1].to_broadcast([P, 512]),
        )

    def v_unit(tj):
        pv = psum.tile([P, 512], F32, tag="mm", bufs=2, name="pv")
        for k in range(KC):
            nc.tensor.matmul(
                pv[:, : 2 * P],
                lhsT=xts[:, k, ts(tj, P)],
                rhs=wvs[:, k, :],
                start=(k == 0),
                stop=(k == KC - 1),
            )
        nc.vector.tensor_copy(
            out=vaug4[:, tj, :, 0:DK],
            in_=pv[:, : 2 * P].rearrange("p (h d) -> p h d", d=DK),
        )

    def proj_unit(tj, msel=None):
        pps = [
            psum.tile([P, 512], F32, tag="mm", bufs=2, name=f"pp{n}")
            for n in range(2)
        ]
        kcs = (0, 1) if msel is None else (msel,)
        for kc in kcs:
            for n in range(2):
                nc.tensor.matmul(
                    pps[n],
                    lhsT=yts[kc][:, ts(tj, P)],
                    rhs=wps[:, kc, ts(n, 512)],
                    start=(kc == kcs[0]),
                    stop=(kc == kcs[-1]),
                )
        for n in range(2):
            ostg = temps.tile([P, 512], BF, tag="ostg", bufs=4, name="ostg")
            nc.vector.tensor_copy(out=ostg, in_=pps[n])
            eng = nc.sync if n == 0 else nc.gpsimd
            if msel == 1:
                # second head-pair's partial goes to its own tensor; the
                # host adds it (avoids write-after-write ordering on `out`)
                eng.dma_start(
                    out=outx[ts(tj - 12, P), ts(n, 512)], in_=ostg
                )
            else:
                eng.dma_start(out=out[ts(tj, P), ts(n, 512)], in_=ostg)

    # FIFO of (name, fn, pe_ns); forced-emitted for deps, else deficit-paced
    fillers = (
        [(("v", tj), (lambda t: lambda: v_unit(t))(tj), 853) for tj in range(4)]
        + [(("qk", 0, d, 1), (lambda d_: lambda: qk_unit(0, d_, 1))(d), 1707)
           for d in ("q", "k")]
        + [(("v", tj), (lambda t: lambda: v_unit(t))(tj), 853)
           for tj in range(4, 8)]
        + [(("qk", 1, d, tg), (lambda d_, t_: lambda: qk_unit(1, d_, t_))(d, tg), 1707)
           for tg in (0, 1) for d in ("q", "k")]
        + [(("qk", 0, d, 2), (lambda d_: lambda: qk_unit(0, d_, 2))(d), 1707)
           for d in ("q", "k")]
        + [(("v", tj), (lambda t: lambda: v_unit(t))(tj), 853)
           for tj in range(8, 12)]
        + [(("qk", 1, d, 2), (lambda d_: lambda: qk_unit(1, d_, 2))(d), 1707)
           for d in ("q", "k")]
        + [(("qk", 0, d, 3), (lambda d_: lambda: qk_unit(0, d_, 3))(d), 1707)
           for d in ("q", "k")]
        + [(("v", tj), (lambda t: lambda: v_unit(t))(tj), 853)
           for tj in range(12, 16)]
        + [(("qk", 1, d, 3), (lambda d_: lambda: qk_unit(1, d_, 3))(d), 1707)
           for d in ("q", "k")]
    )
    emitted = set()
    state = {"deficit": 0.0}

    def pop_filler():
        name, fn, cost = fillers.pop(0)
        fn()
        emitted.add(name)
        state["deficit"] -= cost

    def force(names):
        while any(n not in emitted for n in names):
            pop_filler()

    # bootstrap: first attention group's QK
    qk_unit(0, "q", 0)
    emitted.add(("qk", 0, "q", 0))
    qk_unit(0, "k", 0)
    emitted.add(("qk", 0, "k", 0))

    # ---- attention steps ----
    steps = []
    for m, g in ORDER:
        njc = 4 * g + 4
        for j in range(njc):
            steps.append((m, g, j, njc))

    po_tiles = {}       # (m, g) -> [po_h0, po_h1]
    pt_tiles = {}       # step idx -> pt2 tile
    finalized = set()   # g values with one m done

    def emit_S(i):
        m, g, j, njc = steps[i]
        jrel = j - 4 * g
        band = jrel >= 0
        ncols = 512 - 128 * jrel if band else 512
        qoff = g * 512 + (128 * jrel if band else 0)
        ps2 = psum.tile([P, 1024], F32, tag="s", bufs=2, name="ps2")
        for h in range(2):
            nc.tensor.matmul(
                ps2[:, h * 512 : h * 512 + ncols],
                lhsT=kt[h * DK : (h + 1) * DK, m, ts(j, P)],
                rhs=qt[h * DK : (h + 1) * DK, m, ds(qoff, ncols)],
                start=True,
                stop=True,
                tile_position=(h * DK, 0),
            )
        pt2 = ppool.tile([P, 1024], BF, tag="pt", bufs=4, name="pt2")
        psv = ps2.rearrange("p (h c) -> p h c", h=2)
        ptv = pt2.rearrange("p (h c) -> p h c", h=2)
        nc.scalar.activation(
            ptv[:, :, 0:ncols],
            psv[:, :, 0:ncols],
            mybir.ActivationFunctionType.Exp,
        )
        if band:
            # zero the masked (upper-tri) part of the diagonal 128-block
            nc.vector.tensor_mul(
                out=ptv[:, :, 0:P], in0=ptv[:, :, 0:P], in1=tris
            )
        pt_tiles[i] = pt2

    def emit_PV(i):
        m, g, j, njc = steps[i]
        jrel = j - 4 * g
        band = jrel >= 0
        ncols = 512 - 128 * jrel if band else 512
        if j == 0:
            po_tiles[(m, g)] = [
                psum.tile([DK + 1, 512], F32, tag=f"o{h}", bufs=1, name=f"po{h}")
                for h in range(2)
            ]
        po = po_tiles[(m, g)]
        pt2 = pt_tiles.pop(i)
        co = 128 * jrel if band else 0
        for h in range(2):
            nc.tensor.matmul(
                po[h][:, co : co + ncols],
                lhsT=vaug4[:, j, 2 * m + h, : DK + 1],
                rhs=pt2[:, h * 512 : h * 512 + ncols],
                start=(j == 0),
                stop=(j == njc - 1),
                skip_group_check=True,
            )
        if j == njc - 1:
            finalize(m, g)

    def finalize(m, g):
        po = po_tiles.pop((m, g))
        rbl = temps.tile([DK, 1024], F32, tag="rbl", bufs=2, name="rbl")
        ocs = []
        for h in range(2):
            oc = temps.tile([DK + 1, 512], F32, tag=f"oc{h}", bufs=2, name="oc")
            nc.vector.tensor_copy(out=oc, in_=po[h])
            ocs.append(oc)
            dscr = dpool.tile([512], F32, tag=f"dscr{h}", bufs=4, name="dscr")
            nc.sync.dma_start(out=dscr, in_=oc[DK : DK + 1, :])
            nc.gpsimd.dma_start(
                out=rbl[:, ts(h, 512)],
                in_=bass.AP(
                    tensor=dscr.tensor,
                    offset=dscr.offset,
                    ap=[[0, DK]] + list(dscr.ap),
                ),
            )
        rb = temps.tile([DK, 1024], F32, tag="rb", bufs=2, name="rb")
        nc.vector.reciprocal_approx_fast(out=rb, in_=rbl)
        for h in range(2):
            stg = temps.tile([DK, 512], BF, tag=f"stg{h}", bufs=2, name="stg")
            nc.gpsimd.tensor_mul(
                out=stg, in0=ocs[h][0:DK, :], in1=rb[:, ts(h, 512)]
            )
            nc.gpsimd.dma_start(
                out=yts[m][h * DK : (h + 1) * DK, ts(g, 512)], in_=stg
            )
        # g3 ends the schedule: emit per-m proj halves immediately so they
        # fill the (1,3) pipeline instead of piling up at the kernel tail.
        if g == 3:
            for tj in range(4 * g, 4 * g + 4):
                fillers.append(
                    (("projm", m, tj),
                     (lambda m_, t: lambda: proj_unit(t, m_))(m, tj), 427)
                )
        elif g in finalized:
            for tj in range(4 * g, 4 * g + 4):
                fillers.append(
                    (("proj", tj), (lambda t: lambda: proj_unit(t))(tj), 853)
                )
        else:
            finalized.add(g)

    def s_deps(i):
        m, g, j, njc = steps[i]
        return [("qk", m, "q", g)] + [("qk", m, "k", tg) for tg in range(g + 1)]

    def pv_deps(i):
        m, g, j, njc = steps[i]
        return [("v", j)]

    LOOKAHEAD = 2
    for i in range(len(steps)):
        m, g, j, njc = steps[i]
        force(s_deps(i))
        emit_S(i)
        jrel = j - 4 * g
        ncols = 512 - 128 * jrel if jrel >= 0 else 512
        state["deficit"] += (2 * ncols * 1.23 + 90) - (4 * ncols * 0.4167)
        state["deficit"] = max(-4000.0, min(9000.0, state["deficit"]))
        while state["deficit"] > 900 and fillers:
            pop_filler()
        if i >= LOOKAHEAD:
            force(pv_deps(i - LOOKAHEAD))
            emit_PV(i - LOOKAHEAD)
    for i in range(len(steps) - LOOKAHEAD, len(steps)):
        force(pv_deps(i))
        emit_PV(i)
    while fillers:
        pop_filler()


def _dedupe_ldweights(nc):
    """Drop an InstLdweights when the immediately-preceding PE weight load in
    the scheduled stream is byte-identical (only matmuls in between — they
    don't disturb the stationary operand)."""
    removed = 0
    for f in nc.m.functions:
        for bb in f.blocks:
            insts = list(bb.instructions)
            last_sig = None
            to_remove = []
            for inst in insts:
                tn = type(inst).__name__
                if tn == "InstLdweights":
                    si = inst.sync_info
                    has_sync = si is not None and (
                        list(si.on_wait) or list(si.on_update)
                    )
                    sig = (
                        str(inst.ins[0]),
                        str(inst.tile_position),
                        str(inst.tile_size),
                        str(inst.perf_mode),
                        str(inst.is_transpose),
                    )
                    if sig == last_sig and not has_sync:
                        to_remove.append(inst)
                        continue
                    last_sig = sig
                elif tn == "InstMatmult":
                    continue
                elif getattr(inst, "engine", None) == mybir.EngineType.PE:
                    last_sig = None
            for inst in to_remove:
                bb.instructions.remove(inst)
                removed += 1
    return removed


_NC_CACHE = None


def build():
    global _NC_CACHE
    if _NC_CACHE is not None:
        return _NC_CACHE
    nc = bacc.Bacc("TRN2", target_bir_lowering=False, debug=False, num_devices=8)
    aps = {
        "xt": nc.dram_tensor("xt", [NTG, P, KC, 512], BF, kind="ExternalInput").ap(),
        "wq": nc.dram_tensor("wq", [2, P, KC, P], BF, kind="ExternalInput").ap(),
        "wk": nc.dram_tensor("wk", [2, P, KC, P], BF, kind="ExternalInput").ap(),
        "wv": nc.dram_tensor("wv", [P, KC, 2 * P], BF, kind="ExternalInput").ap(),
        "bq": nc.dram_tensor("bq", [2 * P], F32, kind="ExternalInput").ap(),
        "bk": nc.dram_tensor("bk", [2 * P], F32, kind="ExternalInput").ap(),
        "wp": nc.dram_tensor("wp", [P, 2, C], BF, kind="ExternalInput").ap(),
        "tri": nc.dram_tensor("tri", [P, 2, P], BF, kind="ExternalInput").ap(),
        "out": nc.dram_tensor("out", [T, C], BF, kind="ExternalOutput").ap(),
        "outx": nc.dram_tensor("outx", [512, C], BF, kind="ExternalOutput").ap(),
    }
    with tile.TileContext(nc) as tc:
        with ExitStack() as ctx:
            _emit(ctx, tc, aps)
    _dedupe_ldweights(nc)
    nc.compile()
    _NC_CACHE = nc
    return nc


def make_in_maps(x, Wqkv, bqkv, Wproj):
    """Host-side sharding/layout prep. Returns per-core input dicts."""
    bf = ml_dtypes.bfloat16
    scale = np.float32(1.0 / np.sqrt(DK))
    # post-exp multiplicative causal mask for the diagonal 128-block:
    # tri[k, h, q] = 1 if q >= k else 0 (same for both heads)
    tri1 = (np.arange(P)[None, :] >= np.arange(P)[:, None]).astype(np.float32)
    triv = np.ascontiguousarray(
        np.broadcast_to(tri1[:, None, :], (P, 2, P))
    ).astype(bf)

    def lay_w(w):  # [C, 256] -> [m, p, k, 128] linear
        return np.ascontiguousarray(
            w.reshape(KC, P, 2, P).transpose(2, 1, 0, 3)
        ).astype(bf)

    def lay_x(xb):  # [T, C] -> [tg, p, k, 512] linear
        xt = xb.T  # [C, T]
        return np.ascontiguousarray(
            xt.reshape(KC, P, NTG, 512).transpose(2, 1, 0, 3)
        ).astype(bf)

    xts = [lay_x(x[b]) for b in range(B)]
    in_maps = []
    for c in range(8):
        b, hg = divmod(c, 4)
        lo = hg * HPC * DK
        sl = slice(lo, lo + HPC * DK)
        in_maps.append(
            {
                "xt": xts[b],
                "wq": lay_w(Wqkv[:, 0 * C :][:, sl] * scale),
                "wk": lay_w(Wqkv[:, 1 * C :][:, sl]),
                "wv": np.ascontiguousarray(
                    Wqkv[:, 2 * C :][:, sl].reshape(KC, P, 2 * P).transpose(1, 0, 2)
                ).astype(bf),
                "bq": np.ascontiguousarray(bqkv[0 * C :][sl] * scale).astype(np.float32),
                "bk": np.ascontiguousarray(bqkv[1 * C :][sl]).astype(np.float32),
                "wp": np.ascontiguousarray(
                    Wproj[sl, :].reshape(2, P, C).transpose(1, 0, 2)
                ).astype(bf),
                "tri": triv,
            }
        )
    return in_maps


def gather(outs, bqkv, Wproj, bproj):
    """Sum per-core bf16 partials per batch; fold V-bias + proj-bias."""
    bv = bqkv[2 * C :].astype(np.float32)
    bp_eff = (bproj.astype(np.float32) + bv @ Wproj.astype(np.float32)).astype(
        np.float32
    )
    y = np.empty((B, T, C), np.float32)
    for b in range(B):
        acc = outs[b * 4 + 0][0].astype(np.float32)
        acc[3 * 512 :] += outs[b * 4 + 0][1].astype(np.float32)
        for hg in range(1, 4):
            acc = acc + outs[b * 4 + hg][0].astype(np.float32)
            acc[3 * 512 :] += outs[b * 4 + hg][1].astype(np.float32)
        y[b] = acc + bp_eff[None, :]
    return y


def kernel(x, Wqkv, bqkv, Wproj, bproj):
    global LAST_RESULTS
    x = np.asarray(x, dtype=np.float32)
    Wqkv = np.asarray(Wqkv, dtype=np.float32)
    bqkv = np.asarray(bqkv, dtype=np.float32)
    Wproj = np.asarray(Wproj, dtype=np.float32)
    bproj = np.asarray(bproj, dtype=np.float32)

    nc = build()
    in_maps = make_in_maps(x, Wqkv, bqkv, Wproj)
    try:
        res = bass_utils.run_bass_kernel_spmd(
            nc,
            in_maps,
            core_ids=list(range(8)),
            trace=TRACE,
            **TRACE_KWARGS,
        )
    except Exception:
        if not TRACE:
            raise
        import traceback

        traceback.print_exc()
        print("traced run failed; retrying without trace", file=sys.stderr)
        res = bass_utils.run_bass_kernel_spmd(nc, in_maps, core_ids=list(range(8)))
    LAST_RESULTS = res
    outs = [
        (res.results[c]["out"], res.results[c]["outx"]) for c in range(8)
    ]
    return gather(outs, bqkv, Wproj, bproj)


# revision 25
# speedup vs baseline: 1.2076x; 1.0947x over previous
"""Causal self-attention (B=2, T=2048, C=1024, H=16) on 8 TRN2 NeuronCores.

Sharding: core c -> batch b = c//4, head group hg = c%4 (4 heads/core).
Each core computes QKV for its 4 heads (column-parallel), causal attention,
and a row-parallel partial output projection [T, C] in bf16. The host sums
the 4 partials per batch and adds the analytically-folded biases.

Schedule: a software-pipelined attention j-loop (S matmul + exp run 2 steps
ahead of PV), with QKV/V/proj matmuls deficit-paced as fillers between
attention steps so the PE stays fed while the Scalar engine (exp) streams.
Causal mask is a post-exp multiply by a 0/1 triangle on GpSimd.
"""

import sys

if "/opt/trn_rl_repo" not in sys.path:
    sys.path.insert(0, "/opt/trn_rl_repo")

import numpy as np
import ml_dtypes
from contextlib import ExitStack

import concourse.bass as bass
import concourse.mybir as mybir
import concourse.tile as tile
from concourse import bacc, bass_utils
from concourse.bass import ds, ts


BF = mybir.dt.bfloat16
F32 = mybir.dt.float32

B, T, C = 2, 2048, 1024
H, DK = 16, 64
P = 128
KC = C // P          # 8 contraction chunks over C
NTG = T // 512       # 4 t-groups of 512
NTJ = T // 128       # 16 t-chunks of 128
HPC = 4              # heads per core
VS = 66              # vaug per-head stride (cols 0-63 V, 64 ones, 65 pad)

# interleaved (m, g) order: m0 runs ahead so m1's QK weights can stream in
ORDER = [(0, 0), (0, 1), (1, 0), (1, 1), (0, 2), (1, 2), (0, 3), (1, 3)]

# module-level knobs for test harness
TRACE = False
TRACE_KWARGS = {}
LAST_RESULTS = None


def _emit(ctx, tc, aps):
    nc = tc.nc
    xt, wq, wk, wv, bq, bk, wp, tri, out, outx = (
        aps["xt"], aps["wq"], aps["wk"], aps["wv"], aps["bq"], aps["bk"],
        aps["wp"], aps["tri"], aps["out"], aps["outx"],
    )

    consts = ctx.enter_context(tc.tile_pool(name="consts", bufs=1))
    bigs = ctx.enter_context(tc.tile_pool(name="bigs", bufs=1))
    temps = ctx.enter_context(tc.tile_pool(name="temps", bufs=2))
    ppool = ctx.enter_context(tc.tile_pool(name="ppool", bufs=4))
    psum = ctx.enter_context(tc.tile_pool(name="psum", bufs=1, space="PSUM"))
    dpool = ctx.enter_context(tc.tile_pool(name="dpool", bufs=4, space="DRAM"))

    # ---- SBUF residents ----
    wqs = consts.tile([P, 2, KC, P], BF)
    wks = consts.tile([P, 2, KC, P], BF)
    wvs = consts.tile([P, KC, 2 * P], BF)
    wps = consts.tile([P, 2, C], BF)
    bqs = consts.tile([P, 2], F32)
    bks = consts.tile([P, 2], F32)
    tris = consts.tile([P, 2, P], BF)
    xts = bigs.tile([P, KC, T], BF)
    qt = bigs.tile([P, 2, T], BF)
    kt = bigs.tile([P, 2, T], BF)
    vaug = bigs.tile([P, NTJ, HPC * VS], BF)
    vaug4 = vaug.rearrange("p t (h c) -> p t h c", c=VS)
    yts = [bigs.tile([P, T], BF, name=f"yt{m}") for m in range(2)]

    # ---- input DMAs, ordered so the first QKV matmuls unblock earliest.
    # Two queues (sync / gpsimd); each dma_start stripes over the HW engines.
    # wave 1: everything the bootstrap (qk m0 tg0 + v0-3 + first S) needs,
    # spread over five engine queues so HW DMA engines all engage at once
    nc.sync.dma_start(out=bqs, in_=bq.rearrange("(m p) -> p m", p=P))
    nc.scalar.dma_start(out=wqs[:, 0], in_=wq[0])
    nc.sync.dma_start(out=xts[:, 0:4, 0:512], in_=xt[0, :, 0:4, :])
    nc.gpsimd.dma_start(out=wks[:, 0], in_=wk[0])
    nc.gpsimd.dma_start(out=xts[:, 4:8, 0:512], in_=xt[0, :, 4:8, :])
    nc.scalar.dma_start(out=wvs, in_=wv)
    nc.gpsimd.dma_start(out=tris, in_=tri)
    nc.gpsimd.dma_start(out=bks, in_=bk.rearrange("(m p) -> p m", p=P))
    # wave 2: the rest, in need order, alternating sync/gpsimd
    nc.sync.dma_start(out=xts[:, 0:4, 512:1024], in_=xt[1, :, 0:4, :])
    nc.gpsimd.dma_start(out=xts[:, 4:8, 512:1024], in_=xt[1, :, 4:8, :])
    nc.sync.dma_start(out=wqs[:, 1], in_=wq[1])
    nc.gpsimd.dma_start(out=wks[:, 1], in_=wk[1])
    nc.sync.dma_start(out=xts[:, :, ts(2, 512)], in_=xt[2])
    nc.gpsimd.dma_start(out=xts[:, :, ts(3, 512)], in_=xt[3])
    nc.sync.dma_start(out=wps, in_=wp)

    nc.vector.memset(vaug4[:, :, :, DK : DK + 1], 1.0)

    # ---- filler units (QKV / V / proj work woven between attention steps) ----
    def qk_unit(m, dst, tg):
        wsrc, bsrc, dstt = (
            (wqs, bqs, qt) if dst == "q" else (wks, bks, kt)
        )
        pq = psum.tile([P, 512], F32, tag="mm", bufs=2, name="pq")
        for k in range(KC):
            nc.tensor.matmul(
                pq,
                lhsT=wsrc[:, m, k, :],
                rhs=xts[:, k, ts(tg, 512)],
                start=(k == 0),
                stop=(k == KC - 1),
            )
        nc.vector.tensor_add(
            out=dstt[:, m, ts(tg, 512)],
            in0=pq,
            in1=bsrc[:, m : m + 1].to_broadcast([P, 512]),
        )

    def v_unit(tj):
        pv = psum.tile([P, 512], F32, tag="mm", bufs=2, name="pv")
        for k in range(KC):
            nc.tensor.matmul(
                pv[:, : 2 * P],
                lhsT=xts[:, k, ts(tj, P)],
                rhs=wvs[:, k, :],
                start=(k == 0),
                stop=(k == KC - 1),
            )
        nc.vector.tensor_copy(
            out=vaug4[:, tj, :, 0:DK],
            in_=pv[:, : 2 * P].rearrange("p (h d) -> p h d", d=DK),
        )

    def proj_unit(tj, msel=None):
        pps = [
            psum.tile([P, 512], F32, tag="mm", bufs=2, name=f"pp{n}")
            for n in range(2)
        ]
        kcs = (0, 1) if msel is None else (msel,)
        for kc in kcs:
            for n in range(2):
                nc.tensor.matmul(
                    pps[n],
                    lhsT=yts[kc][:, ts(tj, P)],
                    rhs=wps[:, kc, ts(n, 512)],
                    start=(kc == kcs[0]),
                    stop=(kc == kcs[-1]),
                )
        for n in range(2):
            ostg = temps.tile([P, 512], BF, tag="ostg", bufs=4, name="ostg")
            # alternate the psum drain between DVE and Scalar (Copy shares
            # Exp's act table, so no table reloads) to halve drain latency
            if n == 0:
                nc.vector.tensor_copy(out=ostg, in_=pps[n])
            else:
                nc.scalar.copy(out=ostg, in_=pps[n])
            if msel == 1:
                # second head-pair's partial goes to its own tensor; the
                # host adds it (avoids write-after-write ordering on `out`)
                nc.sync.dma_start(
                    out=outx[ts(tj - 12, P), ts(n, 512)], in_=ostg
                )
            else:
                eng = nc.sync if n == 0 else nc.gpsimd
                eng.dma_start(out=out[ts(tj, P), ts(n, 512)], in_=ostg)

    # FIFO of (name, fn, pe_ns); forced-emitted for deps, else deficit-paced
    fillers = (
        [(("v", tj), (lambda t: lambda: v_unit(t))(tj), 853) for tj in range(4)]
        + [(("qk", 0, d, 1), (lambda d_: lambda: qk_unit(0, d_, 1))(d), 1707)
           for d in ("q", "k")]
        + [(("v", tj), (lambda t: lambda: v_unit(t))(tj), 853)
           for tj in range(4, 8)]
        + [(("qk", 1, d, tg), (lambda d_, t_: lambda: qk_unit(1, d_, t_))(d, tg), 1707)
           for tg in (0, 1) for d in ("q", "k")]
        + [(("qk", 0, d, 2), (lambda d_: lambda: qk_unit(0, d_, 2))(d), 1707)
           for d in ("q", "k")]
        + [(("v", tj), (lambda t: lambda: v_unit(t))(tj), 853)
           for tj in range(8, 12)]
        + [(("qk", 1, d, 2), (lambda d_: lambda: qk_unit(1, d_, 2))(d), 1707)
           for d in ("q", "k")]
        + [(("qk", 0, d, 3), (lambda d_: lambda: qk_unit(0, d_, 3))(d), 1707)
           for d in ("q", "k")]
        + [(("v", tj), (lambda t: lambda: v_unit(t))(tj), 853)
           for tj in range(12, 16)]
        + [(("qk", 1, d, 3), (lambda d_: lambda: qk_unit(1, d_, 3))(d), 1707)
           for d in ("q", "k")]
    )
    emitted = set()
    state = {"deficit": 0.0}

    def pop_filler():
        name, fn, cost = fillers.pop(0)
        fn()
        emitted.add(name)
        state["deficit"] -= cost

    def force(names):
        while any(n not in emitted for n in names):
            pop_filler()

    # bootstrap: first attention group's QK
    qk_unit(0, "q", 0)
    emitted.add(("qk", 0, "q", 0))
    qk_unit(0, "k", 0)
    emitted.add(("qk", 0, "k", 0))

    # ---- attention steps ----
    steps = []
    for m, g in ORDER:
        njc = 4 * g + 4
        for j in range(njc):
            steps.append((m, g, j, njc))

    po_tiles = {}       # (m, g) -> [po_h0, po_h1]
    pt_tiles = {}       # step idx -> pt2 tile
    finalized = set()   # g values with one m done

    def emit_S(i):
        m, g, j, njc = steps[i]
        jrel = j - 4 * g
        band = jrel >= 0
        ncols = 512 - 128 * jrel if band else 512
        qoff = g * 512 + (128 * jrel if band else 0)
        ps2 = psum.tile([P, 1024], F32, tag="s", bufs=2, name="ps2")
        for h in range(2):
            nc.tensor.matmul(
                ps2[:, h * 512 : h * 512 + ncols],
                lhsT=kt[h * DK : (h + 1) * DK, m, ts(j, P)],
                rhs=qt[h * DK : (h + 1) * DK, m, ds(qoff, ncols)],
                start=True,
                stop=True,
                tile_position=(h * DK, 0),
            )
        pt2 = ppool.tile([P, 1024], BF, tag="pt", bufs=4, name="pt2")
        psv = ps2.rearrange("p (h c) -> p h c", h=2)
        ptv = pt2.rearrange("p (h c) -> p h c", h=2)
        nc.scalar.activation(
            ptv[:, :, 0:ncols],
            psv[:, :, 0:ncols],
            mybir.ActivationFunctionType.Exp,
        )
        pt_tiles[i] = pt2

    def emit_tri(i):
        # zero the masked (upper-tri) part of the diagonal 128-block.
        # Emitted AFTER the deficit pops so this DVE op (which waits on the
        # exp) never head-of-line-blocks the fillers' psum drains.
        m, g, j, njc = steps[i]
        if j - 4 * g < 0:
            return
        ptv = pt_tiles[i].rearrange("p (h c) -> p h c", h=2)
        nc.vector.tensor_mul(
            out=ptv[:, :, 0:P], in0=ptv[:, :, 0:P], in1=tris
        )

    def emit_PV(i):
        m, g, j, njc = steps[i]
        jrel = j - 4 * g
        band = jrel >= 0
        ncols = 512 - 128 * jrel if band else 512
        if j == 0:
            po_tiles[(m, g)] = [
                psum.tile([DK + 1, 512], F32, tag=f"o{h}", bufs=1, name=f"po{h}")
                for h in range(2)
            ]
        po = po_tiles[(m, g)]
        pt2 = pt_tiles.pop(i)
        co = 128 * jrel if band else 0
        for h in range(2):
            nc.tensor.matmul(
                po[h][:, co : co + ncols],
                lhsT=vaug4[:, j, 2 * m + h, : DK + 1],
                rhs=pt2[:, h * 512 : h * 512 + ncols],
                start=(j == 0),
                stop=(j == njc - 1),
                skip_group_check=True,
            )
        if j == njc - 1:
            finalize(m, g)

    def finalize(m, g):
        po = po_tiles.pop((m, g))
        rbl = temps.tile([DK, 1024], F32, tag="rbl", bufs=2, name="rbl")
        ocs = []
        for h in range(2):
            oc = temps.tile([DK + 1, 512], F32, tag=f"oc{h}", bufs=2, name="oc")
            nc.vector.tensor_copy(out=oc, in_=po[h])
            ocs.append(oc)
            dscr = dpool.tile([512], F32, tag=f"dscr{h}", bufs=4, name="dscr")
            nc.sync.dma_start(out=dscr, in_=oc[DK : DK + 1, :])
            nc.gpsimd.dma_start(
                out=rbl[:, ts(h, 512)],
                in_=bass.AP(
                    tensor=dscr.tensor,
                    offset=dscr.offset,
                    ap=[[0, DK]] + list(dscr.ap),
                ),
            )
        rb = temps.tile([DK, 1024], F32, tag="rb", bufs=2, name="rb")
        nc.vector.reciprocal_approx_fast(out=rb, in_=rbl)
        for h in range(2):
            stg = temps.tile([DK, 512], BF, tag=f"stg{h}", bufs=2, name="stg")
            nc.gpsimd.tensor_mul(
                out=stg, in0=ocs[h][0:DK, :], in1=rb[:, ts(h, 512)]
            )
            nc.gpsimd.dma_start(
                out=yts[m][h * DK : (h + 1) * DK, ts(g, 512)], in_=stg
            )
        # g3 ends the schedule: emit per-m proj halves immediately so they
        # fill the (1,3) pipeline instead of piling up at the kernel tail.
        if g == 3:
            for tj in range(4 * g, 4 * g + 4):
                fillers.append(
                    (("projm", m, tj),
                     (lambda m_, t: lambda: proj_unit(t, m_))(m, tj), 427)
                )
        elif g in finalized:
            for tj in range(4 * g, 4 * g + 4):
                fillers.append(
                    (("proj", tj), (lambda t: lambda: proj_unit(t))(tj), 853)
                )
        else:
            finalized.add(g)

    def s_deps(i):
        m, g, j, njc = steps[i]
        return [("qk", m, "q", g)] + [("qk", m, "k", tg) for tg in range(g + 1)]

    def pv_deps(i):
        m, g, j, njc = steps[i]
        return [("v", j)]

    LOOKAHEAD = 2
    for i in range(len(steps)):
        m, g, j, njc = steps[i]
        force(s_deps(i))
        emit_S(i)
        jrel = j - 4 * g
        ncols = 512 - 128 * jrel if jrel >= 0 else 512
        state["deficit"] += (2 * ncols * 1.23 + 90) - (4 * ncols * 0.4167)
        state["deficit"] = max(-4000.0, min(9000.0, state["deficit"]))
        while state["deficit"] > 900 and fillers:
            pop_filler()
        emit_tri(i)
        if i >= LOOKAHEAD:
            force(pv_deps(i - LOOKAHEAD))
            emit_PV(i - LOOKAHEAD)
    for i in range(len(steps) - LOOKAHEAD, len(steps)):
        force(pv_deps(i))
        emit_PV(i)
    while fillers:
        pop_filler()


def _dedupe_ldweights(nc):
    """Drop an InstLdweights when the immediately-preceding PE weight load in
    the scheduled stream is byte-identical (only matmuls in between — they
    don't disturb the stationary operand)."""
    removed = 0
    for f in nc.m.functions:
        for bb in f.blocks:
            insts = list(bb.instructions)
            last_sig = None
            to_remove = []
            for inst in insts:
                tn = type(inst).__name__
                if tn == "InstLdweights":
                    si = inst.sync_info
                    has_sync = si is not None and (
                        list(si.on_wait) or list(si.on_update)
                    )
                    sig = (
                        str(inst.ins[0]),
                        str(inst.tile_position),
                        str(inst.tile_size),
                        str(inst.perf_mode),
                        str(inst.is_transpose),
                    )
                    if sig == last_sig and not has_sync:
                        to_remove.append(inst)
                        continue
                    last_sig = sig
                elif tn == "InstMatmult":
                    continue
                elif getattr(inst, "engine", None) == mybir.EngineType.PE:
                    last_sig = None
            for inst in to_remove:
                bb.instructions.remove(inst)
                removed += 1
    return removed


_NC_CACHE = None


def build():
    global _NC_CACHE
    if _NC_CACHE is not None:
        return _NC_CACHE
    nc = bacc.Bacc("TRN2", target_bir_lowering=False, debug=False, num_devices=8)
    aps = {
        "xt": nc.dram_tensor("xt", [NTG, P, KC, 512], BF, kind="ExternalInput").ap(),
        "wq": nc.dram_tensor("wq", [2, P, KC, P], BF, kind="ExternalInput").ap(),
        "wk": nc.dram_tensor("wk", [2, P, KC, P], BF, kind="ExternalInput").ap(),
        "wv": nc.dram_tensor("wv", [P, KC, 2 * P], BF, kind="ExternalInput").ap(),
        "bq": nc.dram_tensor("bq", [2 * P], F32, kind="ExternalInput").ap(),
        "bk": nc.dram_tensor("bk", [2 * P], F32, kind="ExternalInput").ap(),
        "wp": nc.dram_tensor("wp", [P, 2, C], BF, kind="ExternalInput").ap(),
        "tri": nc.dram_tensor("tri", [P, 2, P], BF, kind="ExternalInput").ap(),
        "out": nc.dram_tensor("out", [T, C], BF, kind="ExternalOutput").ap(),
        "outx": nc.dram_tensor("outx", [512, C], BF, kind="ExternalOutput").ap(),
    }
    with tile.TileContext(nc) as tc:
        with ExitStack() as ctx:
            _emit(ctx, tc, aps)
    _dedupe_ldweights(nc)
    nc.compile()
    _NC_CACHE = nc
    return nc


def make_in_maps(x, Wqkv, bqkv, Wproj):
    """Host-side sharding/layout prep. Returns per-core input dicts."""
    bf = ml_dtypes.bfloat16
    scale = np.float32(1.0 / np.sqrt(DK))
    # post-exp multiplicative causal mask for the diagonal 128-block:
    # tri[k, h, q] = 1 if q >= k else 0 (same for both heads)
    tri1 = (np.arange(P)[None, :] >= np.arange(P)[:, None]).astype(np.float32)
    triv = np.ascontiguousarray(
        np.broadcast_to(tri1[:, None, :], (P, 2, P))
    ).astype(bf)

    def lay_w(w):  # [C, 256] -> [m, p, k, 128] linear
        return np.ascontiguousarray(
            w.reshape(KC, P, 2, P).transpose(2, 1, 0, 3)
        ).astype(bf)

    def lay_x(xb):  # [T, C] -> [tg, p, k, 512] linear
        xt = xb.T  # [C, T]
        return np.ascontiguousarray(
            xt.reshape(KC, P, NTG, 512).transpose(2, 1, 0, 3)
        ).astype(bf)

    xts = [lay_x(x[b]) for b in range(B)]
    in_maps = []
    for c in range(8):
        b, hg = divmod(c, 4)
        lo = hg * HPC * DK
        sl = slice(lo, lo + HPC * DK)
        in_maps.append(
            {
                "xt": xts[b],
                "wq": lay_w(Wqkv[:, 0 * C :][:, sl] * scale),
                "wk": lay_w(Wqkv[:, 1 * C :][:, sl]),
                "wv": np.ascontiguousarray(
                    Wqkv[:, 2 * C :][:, sl].reshape(KC, P, 2 * P).transpose(1, 0, 2)
                ).astype(bf),
                "bq": np.ascontiguousarray(bqkv[0 * C :][sl] * scale).astype(np.float32),
                "bk": np.ascontiguousarray(bqkv[1 * C :][sl]).astype(np.float32),
                "wp": np.ascontiguousarray(
                    Wproj[sl, :].reshape(2, P, C).transpose(1, 0, 2)
                ).astype(bf),
                "tri": triv,
            }
        )
    return in_maps


def gather(outs, bqkv, Wproj, bproj):
    """Sum per-core bf16 partials per batch; fold V-bias + proj-bias."""
    bv = bqkv[2 * C :].astype(np.float32)
    bp_eff = (bproj.astype(np.float32) + bv @ Wproj.astype(np.float32)).astype(
        np.float32
    )
    y = np.empty((B, T, C), np.float32)
    for b in range(B):
        acc = outs[b * 4 + 0][0].astype(np.float32)
        acc[3 * 512 :] += outs[b * 4 + 0][1].astype(np.float32)
        for hg in range(1, 4):
            acc = acc + outs[b * 4 + hg][0].astype(np.float32)
            acc[3 * 512 :] += outs[b * 4 + hg][1].astype(np.float32)
        y[b] = acc + bp_eff[None, :]
    return y


def kernel(x, Wqkv, bqkv, Wproj, bproj):
    global LAST_RESULTS
    x = np.asarray(x, dtype=np.float32)
    Wqkv = np.asarray(Wqkv, dtype=np.float32)
    bqkv = np.asarray(bqkv, dtype=np.float32)
    Wproj = np.asarray(Wproj, dtype=np.float32)
    bproj = np.asarray(bproj, dtype=np.float32)

    nc = build()
    in_maps = make_in_maps(x, Wqkv, bqkv, Wproj)
    try:
        res = bass_utils.run_bass_kernel_spmd(
            nc,
            in_maps,
            core_ids=list(range(8)),
            trace=TRACE,
            **TRACE_KWARGS,
        )
    except Exception:
        if not TRACE:
            raise
        import traceback

        traceback.print_exc()
        print("traced run failed; retrying without trace", file=sys.stderr)
        res = bass_utils.run_bass_kernel_spmd(nc, in_maps, core_ids=list(range(8)))
    LAST_RESULTS = res
    outs = [
        (res.results[c]["out"], res.results[c]["outx"]) for c in range(8)
    ]
    return gather(outs, bqkv, Wproj, bproj)
